# revision 29
# baseline (speedup 1.0000x reference)
"""Trainium2 Bass kernel for the AttentiveModule problem.

Reference computation (per batch element b, S=1024, D=512):
    att   = aspect @ inp.T / sqrt(len)                # [S,S]
    exp   = att * mask[:, None]                       # row mask (query dim)
    att_n = exp / (exp.sum(-1, keepdims=True) + 1e-4) # linear normalize
    w     = att_n @ inp                               # [S,D]
    ffn_inp = w + (inp + aspect) * mask[:, None]
    o1    = relu(ffn_inp @ w1.T + b1)
    o2    = relu(o1 @ w2.T + b2)
    final = 2*ffn_inp + o2
    out   = final / ||final||_2(axis=-1)

Sharding: data-parallel over batch, one batch element per NeuronCore (8 cores).

Key algebraic restructuring:
  - The attention is linear (no softmax), so the row mask and the linear
    normalization reduce to a per-row factor g[s] computed on the host:
      w[s,:] = g[s] * (aspect[s,:] @ inp.T @ inp)
    and by associativity the [S,S] intermediate is never materialized:
      M  = inp.T @ inp                      # [D,D], 32 matmuls
      W2 = (2g*aspect) @ M                  # [S,D], 32 matmuls (row space)
      ffnT2 = M @ (2g*aspect).T             # [D,S], 32 matmuls (col space,
                                            #  M is symmetric) -- this is
                                            #  2*ffn_inp.T, feeding FFN1
                                            #  directly with no PE transposes
    vs 64+64 matmuls + 32 PE transposes for the direct chain.
  - g and the factor 2 of final = 2*ffn_inp + o2 fold into aspect on the
    host; w1 compensates with 0.5.
  - FFN2 bias enters PSUM via a K=1 ones x b2 matmul.
  - Inputs are packed host-side into [128, X] images matching SBUF layout,
    streamed in consumption order across both HWDGE queues (sync + scalar).

Matmul operands are bf16 (fp32 PSUM accumulation); the final residual path
stays fp32.
"""

import os
import sys

for _p in ("/opt/trn_rl_repo", "/opt/pypackages"):
    if os.path.isdir(_p) and _p not in sys.path:
        sys.path.append(_p)

import numpy as np
import ml_dtypes

BF16 = ml_dtypes.bfloat16

B, S, D = 8, 1024, 512
N_CORES = 8
P = 128                     # SBUF partitions
SB = S // P                 # 8 s-blocks of 128
DB = D // P                 # 4 d-blocks of 128
TB = S // P                 # 8 t-blocks of 128
NF = 512                    # matmul moving free dim (one fp32 PSUM bank)
SH = S // NF                # 2 s-halves

# --- packed input layouts (element column offsets) -----------------------
X_COLS = TB * D             # gX  (bf16): x, tb-major           [P, 4096]
A_COLS = DB * S             # gA  (bf16): aTg2 = (2g*a).T, db-major [P, 4096]
RT_COLS = DB * S            # gRT (bf16): resm2T, db-major      [P, 4096]
R_COLS = SB * D             # gR  (bf16): resm2, sb-major       [P, 4096]
W_W1 = 0                    # gW  (bf16): w1th | w2t | b2 row | ones row
W_W2 = DB * D               # 2048
W_B2 = W_W2 + DB * D        # 4096  (row 0 only)
W_ONES = W_B2 + D           # 4608  (row 0 only)
W_COLS = W_ONES + P         # 4736
F_COLS = DB                 # gF  (f32): b1 as [P, DB] columns

_COMPILED = None


def _build():
    import concourse.bacc as bacc
    import concourse.tile as tile
    import concourse.mybir as mybir

    f32 = mybir.dt.float32
    bf16 = mybir.dt.bfloat16
    AF = mybir.ActivationFunctionType
    ALU = mybir.AluOpType

    nc = bacc.Bacc("TRN2", target_bir_lowering=False, debug=False,
                   num_devices=N_CORES)

    packX = nc.dram_tensor("packX", [P, X_COLS], bf16, kind="ExternalInput").ap()
    packA = nc.dram_tensor("packA", [P, A_COLS], bf16, kind="ExternalInput").ap()
    packRT = nc.dram_tensor("packRT", [P, RT_COLS], bf16, kind="ExternalInput").ap()
    packR = nc.dram_tensor("packR", [P, R_COLS], bf16, kind="ExternalInput").ap()
    packW = nc.dram_tensor("packW", [P, W_COLS], bf16, kind="ExternalInput").ap()
    packF = nc.dram_tensor("packF", [P, F_COLS], f32, kind="ExternalInput").ap()
    out = nc.dram_tensor("out", [S, D], f32, kind="ExternalOutput").ap()

    with tile.TileContext(nc) as tc:
        import contextlib
        ctx = contextlib.ExitStack()
        with ctx:
            big = ctx.enter_context(tc.tile_pool(name="big", bufs=1))
            psA = ctx.enter_context(tc.tile_pool(name="psA", bufs=8, space="PSUM"))
            work = ctx.enter_context(tc.tile_pool(name="work", bufs=4))

            # ---- input streams: interleaved across BOTH HWDGE queues in
            # consumption-need order (the head is HBM-bandwidth-bound, so
            # each tensor is split half per queue and ordered by when the
            # compute needs it: X -> A -> RT -> R -> W) --------------------
            gX = big.tile([P, X_COLS], bf16, name="gX")
            gA = big.tile([P, A_COLS], bf16, name="gA")
            gRT = big.tile([P, RT_COLS], bf16, name="gRT")
            gR = big.tile([P, R_COLS], bf16, name="gR")
            gW = big.tile([P, W_COLS], bf16, name="gW")
            gF = big.tile([P, F_COLS], f32, name="gF")

            # X: per-t-block chunks so phase M runs DMA-paced from the
            # start; tb0 goes via the GpSimd SWDGE queue, which clears the
            # framework preamble ~1us before the HWDGE queues
            nc.gpsimd.dma_start(gX[:, 0:D // 2], packX[:, 0:D // 2])
            nc.gpsimd.dma_start(gX[:, D // 2:D], packX[:, D // 2:D])
            for tb in range(1, TB):
                eng = nc.sync if tb % 2 == 0 else nc.scalar
                eng.dma_start(gX[:, tb * D:(tb + 1) * D],
                              packX[:, tb * D:(tb + 1) * D])
            # gA/gRT are packed h-major ([h][block][NF]); stream each
            # h-half split across both queues so phase T h0-groups start
            # as soon as the first half lands
            HB = DB * NF                       # one h-half = 2048 cols
            for h in range(SH):
                lo, mid, hi = h * HB, h * HB + HB // 2, (h + 1) * HB
                nc.sync.dma_start(gA[:, lo:mid], packA[:, lo:mid])
                nc.scalar.dma_start(gA[:, mid:hi], packA[:, mid:hi])
                nc.sync.dma_start(gRT[:, lo:mid], packRT[:, lo:mid])
                nc.scalar.dma_start(gRT[:, mid:hi], packRT[:, mid:hi])
            HR = R_COLS // 2
            nc.sync.dma_start(gR[:, 0:HR], packR[:, 0:HR])
            nc.scalar.dma_start(gR[:, HR:], packR[:, HR:])
            HW_ = W_COLS // 2
            nc.sync.dma_start(gW[:, 0:HW_], packW[:, 0:HW_])
            nc.scalar.dma_start(gW[:, HW_:], packW[:, HW_:])
            nc.scalar.dma_start(gF[:], packF[:])

            # PE warm-up: HAM un-throttles only after ~3.4us of CONTIGUOUS
            # PE busy -- any gap restarts the clock. Memset the operands
            # first so the warm-up starts as early as possible, and make it
            # long enough to bridge gaplessly into phase M's first matmul.
            wls = work.tile([P, P], bf16, name="wls_t", tag="wls")
            nc.gpsimd.memset(wls[:], 0.0)
            wrh = work.tile([P, NF], bf16, name="wrh_t", tag="wrh")
            nc.gpsimd.memset(wrh[:], 0.0)
            wps = psA.tile([P, NF], f32, name="wps_t", tag="psA")
            for _ in range(6):
                nc.tensor.matmul(wps[:], wls[:], wrh[:], start=True, stop=True)

            # force the Square/Sqrt ACT table loads during the DMA-wait head
            warm = work.tile([P, 1], f32, name="warm_t", tag="warm")
            nc.gpsimd.memset(warm[:], 0.0)
            warm2 = work.tile([P, 1], f32, name="warm2_t", tag="warm2")
            nc.scalar.activation(warm2[:], warm[:], AF.Square)
            nc.scalar.activation(warm2[:], warm[:], AF.Sqrt)

            def X(tb):              # [P, D] x rows t-block tb
                return gX[:, tb * D:(tb + 1) * D]

            def w1th(db):           # [P, D]
                return gW[:, W_W1 + db * D: W_W1 + (db + 1) * D]

            def w2t(eb):            # [P, D]
                return gW[:, W_W2 + eb * D: W_W2 + (eb + 1) * D]

            b2row = gW[0:1, W_B2: W_B2 + D]
            onesrow = gW[0:1, W_ONES: W_ONES + P]

            def b1col(eb):          # [P, 1] f32
                return gF[:, eb: eb + 1]

            # ---- phase M: M[d1,d2] = sum_t x[t,d1] x[t,d2]  (4 live banks)
            psm = [psA.tile([P, NF], f32, name=f"psM{db}", tag="psA")
                   for db in range(DB)]
            for tb in range(TB):
                for db in range(DB):
                    nc.tensor.matmul(
                        psm[db][:],
                        X(tb)[:, db * P:(db + 1) * P],
                        X(tb),
                        start=(tb == 0),
                        stop=(tb == TB - 1),
                    )
            # evacuate on DVE in do-column halves so phase T's first groups
            # (which read the low do-columns of every block) unblock early;
            # GpSimd cannot read PSUM and Scalar is busy with table loads
            Mb = big.tile([P, DB * NF], bf16, name="Mb")   # M[kb-rows, :]
            HN = NF // 2
            for half in range(2):
                for db in range(DB):
                    nc.vector.tensor_copy(
                        Mb[:, db * NF + half * HN: db * NF + (half + 1) * HN],
                        psm[db][:, half * HN:(half + 1) * HN])

            # ---- phase T: ffnT2[do,s] = sum_di M[di,do] aTg2[di,s] + resm2T
            gFT = big.tile([P, DB * S], bf16, name="gFT")  # do-major [do*S+s]
            for h in range(SH):
                for do in range(DB):
                    ps = psA.tile([P, NF], f32, name="psT_t", tag="psA")
                    for kb in range(DB):
                        nc.tensor.matmul(
                            ps[:],
                            Mb[:, kb * NF + do * P: kb * NF + (do + 1) * P],
                            gA[:, h * HB + kb * NF: h * HB + (kb + 1) * NF],
                            start=(kb == 0),
                            stop=(kb == DB - 1),
                        )
                    nc.vector.tensor_add(
                        gFT[:, do * S + h * NF: do * S + (h + 1) * NF],
                        ps[:],
                        gRT[:, h * HB + do * NF: h * HB + (do + 1) * NF])

            # ---- phase W: W2[s,do] = sum_di aTg2[di,s] M[di,do]; F2=W2+resm2
            F2_sb = []
            for sb in range(SB):
                f2 = big.tile([P, D], f32, name=f"F2_sb{sb}")
                F2_sb.append(f2)
            for sb in range(SB):
                ps = psA.tile([P, NF], f32, name="psW_t", tag="psA")
                hs, sw = sb // 4, sb % 4
                for kb in range(DB):
                    nc.tensor.matmul(
                        ps[:],
                        gA[:, hs * HB + kb * NF + sw * P:
                            hs * HB + kb * NF + (sw + 1) * P],
                        Mb[:, kb * NF:(kb + 1) * NF],
                        start=(kb == 0),
                        stop=(kb == DB - 1),
                    )
                nc.vector.tensor_add(F2_sb[sb][:], ps[:],
                                     gR[:, sb * D:(sb + 1) * D])

            # ---- phase D: o1T = relu(w1th.T @ ffnT2 + b1)  [e, s] ---------
            o1T_eb = []
            for eb in range(DB):
                t = big.tile([P, S], bf16, name=f"o1T_eb{eb}")
                o1T_eb.append(t)

            def d_group(h, eb):
                ps = psA.tile([P, NF], f32, name="psD_t", tag="psA")
                for db in range(DB):
                    nc.tensor.matmul(
                        ps[:],
                        w1th(db)[:, eb * P:(eb + 1) * P],
                        gFT[:, db * S + h * NF: db * S + (h + 1) * NF],
                        start=(db == 0),
                        stop=(db == DB - 1),
                    )
                nc.scalar.activation(
                    o1T_eb[eb][:, h * NF:(h + 1) * NF], ps[:], AF.Relu,
                    bias=b1col(eb), scale=1.0)

            # ---- phase E: o2, final, normalize, store --------------------
            def emit_ot_store(sb, fin, rr):
                ot = work.tile([P, D], f32, name="ot_t", tag="ot")
                nc.vector.tensor_scalar_mul(ot[:], fin[:], rr[:])
                if sb < SB - 1:
                    nc.sync.dma_start(out[sb * P:(sb + 1) * P, :], ot[:])
                else:
                    # final block: two half stores stream in parallel
                    HD = D // 2
                    nc.sync.dma_start(out[sb * P:(sb + 1) * P, :HD],
                                      ot[:, :HD])
                    nc.scalar.dma_start(out[sb * P:(sb + 1) * P, HD:],
                                        ot[:, HD:])

            # ot+store for group k is emitted ~3 groups late: DVE is FIFO,
            # so the stt (which recycles the PSUM bank) stays ahead of the
            # deferrable scale work
            pend = []

            def e_group(sb):
                # bias matmul last: a K=128 matmul leads the group so its
                # LDWEIGHTS prefetches during the previous group's tail
                ps = psA.tile([P, NF], f32, name="psE_t", tag="psA")
                for eb in range(DB):
                    nc.tensor.matmul(
                        ps[:],
                        o1T_eb[eb][:, sb * P:(sb + 1) * P],
                        w2t(eb),
                        start=(eb == 0),
                        stop=False,
                    )
                nc.tensor.matmul(ps[:], onesrow, b2row, start=False, stop=True)
                # epilogue: fin bf16 (~1e-3 rel err); DVE: stt+recip+ot,
                # ACT: square+sqrt
                fin = work.tile([P, D], bf16, name="fin_t", tag="fin")
                sq = work.tile([P, D], bf16, name="sq_t", tag="sq")
                ss = work.tile([P, 1], f32, name="ss_t", tag="ss")
                nc.vector.scalar_tensor_tensor(
                    fin[:], ps[:], 0.0, F2_sb[sb][:], ALU.max, ALU.add)
                nc.scalar.activation(sq[:], fin[:], AF.Square,
                                     accum_out=ss[:])
                rn = work.tile([P, 1], f32, name="rn_t", tag="rn")
                nc.scalar.activation(rn[:], ss[:], AF.Sqrt)
                rr = work.tile([P, 1], f32, name="rr_t", tag="rr")
                nc.vector.reciprocal(rr[:], rn[:])
                pend.append((sb, fin, rr))
                if len(pend) > 3:
                    emit_ot_store(*pend.pop(0))

            # interleave: D-h0, E sb0-3 (o1T columns live in h0), D-h1,
            # E sb4-7 -- spreads the epilogue over D-h1's PE time
            for eb in range(DB):
                d_group(0, eb)
            for sb in range(4):
                e_group(sb)
            for eb in range(DB):
                d_group(1, eb)
            for sb in range(4, SB - 1):
                e_group(sb)
            # drain deferred stores while sb7's matmuls run, so the final
            # chain (stt7 -> ... -> ot7 -> store) isn't queued behind them
            for args in pend:
                emit_ot_store(*args)
            pend.clear()
            e_group(SB - 1)
            for args in pend:
                emit_ot_store(*args)

    nc.compile()
    return nc


def _get_compiled():
    global _COMPILED
    if _COMPILED is None:
        _COMPILED = _build()
    return _COMPILED


def _host_prep(inp, inp_len, aspect, w1, b1, w2, b2):
    inp = np.asarray(inp, dtype=np.float32)
    aspect = np.asarray(aspect, dtype=np.float32)
    inp_len = np.asarray(inp_len, dtype=np.float32)
    w1 = np.asarray(w1, dtype=np.float32)
    b1 = np.asarray(b1, dtype=np.float32)
    w2 = np.asarray(w2, dtype=np.float32)
    b2 = np.asarray(b2, dtype=np.float32)

    packW = np.zeros((P, W_COLS), dtype=BF16)
    w1th = (w1.T * 0.5).astype(BF16)                 # [d, e]
    w2tt = w2.T.astype(BF16)                         # [e, f]
    for db in range(DB):
        packW[:, W_W1 + db * D: W_W1 + (db + 1) * D] = \
            w1th[db * P:(db + 1) * P, :]
        packW[:, W_W2 + db * D: W_W2 + (db + 1) * D] = \
            w2tt[db * P:(db + 1) * P, :]
    packW[0, W_B2: W_B2 + D] = b2.astype(BF16)
    packW[0, W_ONES: W_ONES + P] = np.ones(P, dtype=BF16)

    packF = b1.reshape(DB, P).T.astype(np.float32)   # [P, DB]

    in_maps = []
    for bidx in range(B):
        x = inp[bidx].astype(np.float64)             # [S, D]
        a = aspect[bidx].astype(np.float64)
        ln = float(inp_len[bidx])
        scale = np.sqrt(ln)
        mask = (np.arange(S) < int(ln)).astype(np.float64)
        rowsum = a @ x.sum(axis=0)
        g = mask / (mask * rowsum + 1e-4 * scale)
        aTg2 = ((a * (2.0 * g)[:, None]).T).astype(BF16)   # [D, S]
        resm2 = (2.0 * (x + a) * mask[:, None]).astype(BF16)  # [S, D]

        pX = np.empty((P, X_COLS), dtype=BF16)
        xb = x.astype(BF16)
        for tb in range(TB):
            pX[:, tb * D:(tb + 1) * D] = xb[tb * P:(tb + 1) * P, :]

        pA = np.empty((P, A_COLS), dtype=BF16)
        HB = DB * NF
        for h in range(SH):
            for db in range(DB):
                pA[:, h * HB + db * NF:(h * HB + (db + 1) * NF)] = \
                    aTg2[db * P:(db + 1) * P, h * NF:(h + 1) * NF]

        pRT = np.empty((P, RT_COLS), dtype=BF16)
        r2T = resm2.T                                 # [D, S]
        for h in range(SH):
            for db in range(DB):
                pRT[:, h * HB + db * NF:(h * HB + (db + 1) * NF)] = \
                    r2T[db * P:(db + 1) * P, h * NF:(h + 1) * NF]

        pR = np.empty((P, R_COLS), dtype=BF16)
        for sb in range(SB):
            pR[:, sb * D:(sb + 1) * D] = resm2[sb * P:(sb + 1) * P, :]

        in_maps.append({"packX": pX, "packA": pA, "packRT": pRT,
                        "packR": pR, "packW": packW, "packF": packF})
    return in_maps


def kernel(inp, inp_len, aspect, w1, b1, w2, b2):
    from concourse.bass_utils import run_bass_kernel_spmd

    nc = _get_compiled()
    in_maps = _host_prep(inp, inp_len, aspect, w1, b1, w2, b2)
    res = run_bass_kernel_spmd(nc, in_maps, core_ids=list(range(N_CORES)))
    return np.stack([res.results[i]["out"] for i in range(N_CORES)], axis=0)


# revision 30
# speedup vs baseline: 1.0294x; 1.0294x over previous
"""Trainium2 Bass kernel for the AttentiveModule problem.

Reference computation (per batch element b, S=1024, D=512):
    att   = aspect @ inp.T / sqrt(len)                # [S,S]
    exp   = att * mask[:, None]                       # row mask (query dim)
    att_n = exp / (exp.sum(-1, keepdims=True) + 1e-4) # linear normalize
    w     = att_n @ inp                               # [S,D]
    ffn_inp = w + (inp + aspect) * mask[:, None]
    o1    = relu(ffn_inp @ w1.T + b1)
    o2    = relu(o1 @ w2.T + b2)
    final = 2*ffn_inp + o2
    out   = final / ||final||_2(axis=-1)

Sharding: data-parallel over batch, one batch element per NeuronCore (8 cores).

Key algebraic restructuring:
  - The attention is linear (no softmax), so the row mask and the linear
    normalization reduce to a per-row factor g[s] computed on the host:
      w[s,:] = g[s] * (aspect[s,:] @ inp.T @ inp)
    and by associativity the [S,S] intermediate is never materialized:
      M  = inp.T @ inp                      # [D,D], 32 matmuls
      W2 = (2g*aspect) @ M                  # [S,D], 32 matmuls (row space)
      ffnT2 = M @ (2g*aspect).T             # [D,S], 32 matmuls (col space,
                                            #  M is symmetric) -- this is
                                            #  2*ffn_inp.T, feeding FFN1
                                            #  directly with no PE transposes
    vs 64+64 matmuls + 32 PE transposes for the direct chain.
  - g and the factor 2 of final = 2*ffn_inp + o2 fold into aspect on the
    host; w1 compensates with 0.5.
  - FFN2 bias enters PSUM via a K=1 ones x b2 matmul.
  - Inputs are packed host-side into [128, X] images matching SBUF layout,
    streamed in consumption order across both HWDGE queues (sync + scalar).

Matmul operands are bf16 (fp32 PSUM accumulation); the final residual path
stays fp32.
"""

import os
import sys

for _p in ("/opt/trn_rl_repo", "/opt/pypackages"):
    if os.path.isdir(_p) and _p not in sys.path:
        sys.path.append(_p)

import numpy as np
import ml_dtypes

BF16 = ml_dtypes.bfloat16

B, S, D = 8, 1024, 512
N_CORES = 8
P = 128                     # SBUF partitions
SB = S // P                 # 8 s-blocks of 128
DB = D // P                 # 4 d-blocks of 128
TB = S // P                 # 8 t-blocks of 128
NF = 512                    # matmul moving free dim (one fp32 PSUM bank)
SH = S // NF                # 2 s-halves

# --- packed input layouts (element column offsets) -----------------------
X_COLS = TB * D             # gX  (bf16): x, tb-major           [P, 4096]
A_COLS = DB * S             # gA  (bf16): aTg2 = (2g*a).T, db-major [P, 4096]
RT_COLS = DB * S            # gRT (bf16): resm2T, db-major      [P, 4096]
R_COLS = SB * D             # gR  (bf16): resm2, sb-major       [P, 4096]
W_W1 = 0                    # gW  (bf16): w1th | w2t | b2 row | ones row
W_W2 = DB * D               # 2048
W_B2 = W_W2 + DB * D        # 4096  (row 0 only)
W_ONES = W_B2 + D           # 4608  (row 0 only)
W_COLS = W_ONES + P         # 4736
F_COLS = DB                 # gF  (f32): b1 as [P, DB] columns

_COMPILED = None


def _build():
    import concourse.bacc as bacc
    import concourse.tile as tile
    import concourse.mybir as mybir

    f32 = mybir.dt.float32
    bf16 = mybir.dt.bfloat16
    AF = mybir.ActivationFunctionType
    ALU = mybir.AluOpType

    nc = bacc.Bacc("TRN2", target_bir_lowering=False, debug=False,
                   num_devices=N_CORES)

    packX = nc.dram_tensor("packX", [P, X_COLS], bf16, kind="ExternalInput").ap()
    packA = nc.dram_tensor("packA", [P, A_COLS], bf16, kind="ExternalInput").ap()
    packRT = nc.dram_tensor("packRT", [P, RT_COLS], bf16, kind="ExternalInput").ap()
    packR = nc.dram_tensor("packR", [P, R_COLS], bf16, kind="ExternalInput").ap()
    packW = nc.dram_tensor("packW", [P, W_COLS], bf16, kind="ExternalInput").ap()
    packF = nc.dram_tensor("packF", [P, F_COLS], f32, kind="ExternalInput").ap()
    out = nc.dram_tensor("out", [S, D], f32, kind="ExternalOutput").ap()

    with tile.TileContext(nc) as tc:
        import contextlib
        ctx = contextlib.ExitStack()
        with ctx:
            big = ctx.enter_context(tc.tile_pool(name="big", bufs=1))
            psA = ctx.enter_context(tc.tile_pool(name="psA", bufs=8, space="PSUM"))
            work = ctx.enter_context(tc.tile_pool(name="work", bufs=4))

            # ---- input streams: interleaved across BOTH HWDGE queues in
            # consumption-need order (the head is HBM-bandwidth-bound, so
            # each tensor is split half per queue and ordered by when the
            # compute needs it: X -> A -> RT -> R -> W) --------------------
            gX = big.tile([P, X_COLS], bf16, name="gX")
            gA = big.tile([P, A_COLS], bf16, name="gA")
            gRT = big.tile([P, RT_COLS], bf16, name="gRT")
            gR = big.tile([P, R_COLS], bf16, name="gR")
            gW = big.tile([P, W_COLS], bf16, name="gW")
            gF = big.tile([P, F_COLS], f32, name="gF")

            # X: per-t-block chunks so phase M runs DMA-paced from the
            # start; tb0 split across both queues to land soonest
            nc.sync.dma_start(gX[:, 0:D // 2], packX[:, 0:D // 2])
            nc.scalar.dma_start(gX[:, D // 2:D], packX[:, D // 2:D])
            for tb in range(1, TB):
                eng = nc.sync if tb % 2 == 0 else nc.scalar
                eng.dma_start(gX[:, tb * D:(tb + 1) * D],
                              packX[:, tb * D:(tb + 1) * D])
            # gA/gRT are packed h-major ([h][block][NF]); stream each
            # h-half split across both queues so phase T h0-groups start
            # as soon as the first half lands
            HB = DB * NF                       # one h-half = 2048 cols
            for h in range(SH):
                lo, mid, hi = h * HB, h * HB + HB // 2, (h + 1) * HB
                nc.sync.dma_start(gA[:, lo:mid], packA[:, lo:mid])
                nc.scalar.dma_start(gA[:, mid:hi], packA[:, mid:hi])
                nc.sync.dma_start(gRT[:, lo:mid], packRT[:, lo:mid])
                nc.scalar.dma_start(gRT[:, mid:hi], packRT[:, mid:hi])
            HR = R_COLS // 2
            nc.sync.dma_start(gR[:, 0:HR], packR[:, 0:HR])
            nc.scalar.dma_start(gR[:, HR:], packR[:, HR:])
            HW_ = W_COLS // 2
            nc.sync.dma_start(gW[:, 0:HW_], packW[:, 0:HW_])
            nc.scalar.dma_start(gW[:, HW_:], packW[:, HW_:])
            nc.scalar.dma_start(gF[:], packF[:])

            # PE warm-up: HAM un-throttles only after ~3.4us of CONTIGUOUS
            # PE busy -- any gap restarts the clock. Memset the operands
            # first so the warm-up starts as early as possible, and make it
            # long enough to bridge gaplessly into phase M's first matmul.
            wls = work.tile([P, P], bf16, name="wls_t", tag="wls")
            nc.gpsimd.memset(wls[:], 0.0)
            wrh = work.tile([P, NF], bf16, name="wrh_t", tag="wrh")
            nc.gpsimd.memset(wrh[:], 0.0)
            wps = psA.tile([P, NF], f32, name="wps_t", tag="psA")
            for _ in range(7):
                nc.tensor.matmul(wps[:], wls[:], wrh[:], start=True, stop=True)

            # force the Square/Sqrt ACT table loads during the DMA-wait head
            warm = work.tile([P, 1], f32, name="warm_t", tag="warm")
            nc.gpsimd.memset(warm[:], 0.0)
            warm2 = work.tile([P, 1], f32, name="warm2_t", tag="warm2")
            nc.scalar.activation(warm2[:], warm[:], AF.Square)
            nc.scalar.activation(warm2[:], warm[:], AF.Sqrt)

            def X(tb):              # [P, D] x rows t-block tb
                return gX[:, tb * D:(tb + 1) * D]

            def w1th(db):           # [P, D]
                return gW[:, W_W1 + db * D: W_W1 + (db + 1) * D]

            def w2t(eb):            # [P, D]
                return gW[:, W_W2 + eb * D: W_W2 + (eb + 1) * D]

            b2row = gW[0:1, W_B2: W_B2 + D]
            onesrow = gW[0:1, W_ONES: W_ONES + P]

            def b1col(eb):          # [P, 1] f32
                return gF[:, eb: eb + 1]

            # ---- phase M: M[d1,d2] = sum_t x[t,d1] x[t,d2]  (4 live banks)
            psm = [psA.tile([P, NF], f32, name=f"psM{db}", tag="psA")
                   for db in range(DB)]
            for tb in range(TB):
                for db in range(DB):
                    nc.tensor.matmul(
                        psm[db][:],
                        X(tb)[:, db * P:(db + 1) * P],
                        X(tb),
                        start=(tb == 0),
                        stop=(tb == TB - 1),
                    )
            # evacuate on DVE in do-column halves so phase T's first groups
            # (which read the low do-columns of every block) unblock early;
            # GpSimd cannot read PSUM and Scalar is busy with table loads
            Mb = big.tile([P, DB * NF], bf16, name="Mb")   # M[kb-rows, :]
            HN = NF // 2
            for half in range(2):
                for db in range(DB):
                    nc.vector.tensor_copy(
                        Mb[:, db * NF + half * HN: db * NF + (half + 1) * HN],
                        psm[db][:, half * HN:(half + 1) * HN])

            # ---- phase T: ffnT2[do,s] = sum_di M[di,do] aTg2[di,s] + resm2T
            gFT = big.tile([P, DB * S], bf16, name="gFT")  # do-major [do*S+s]
            for h in range(SH):
                for do in range(DB):
                    ps = psA.tile([P, NF], f32, name="psT_t", tag="psA")
                    for kb in range(DB):
                        nc.tensor.matmul(
                            ps[:],
                            Mb[:, kb * NF + do * P: kb * NF + (do + 1) * P],
                            gA[:, h * HB + kb * NF: h * HB + (kb + 1) * NF],
                            start=(kb == 0),
                            stop=(kb == DB - 1),
                        )
                    nc.vector.tensor_add(
                        gFT[:, do * S + h * NF: do * S + (h + 1) * NF],
                        ps[:],
                        gRT[:, h * HB + do * NF: h * HB + (do + 1) * NF])

            # ---- phase W: W2[s,do] = sum_di aTg2[di,s] M[di,do]; F2=W2+resm2
            F2_sb = []
            for sb in range(SB):
                f2 = big.tile([P, D], f32, name=f"F2_sb{sb}")
                F2_sb.append(f2)
            for sb in range(SB):
                ps = psA.tile([P, NF], f32, name="psW_t", tag="psA")
                hs, sw = sb // 4, sb % 4
                for kb in range(DB):
                    nc.tensor.matmul(
                        ps[:],
                        gA[:, hs * HB + kb * NF + sw * P:
                            hs * HB + kb * NF + (sw + 1) * P],
                        Mb[:, kb * NF:(kb + 1) * NF],
                        start=(kb == 0),
                        stop=(kb == DB - 1),
                    )
                nc.vector.tensor_add(F2_sb[sb][:], ps[:],
                                     gR[:, sb * D:(sb + 1) * D])

            # ---- phase D: o1T = relu(w1th.T @ ffnT2 + b1)  [e, s] ---------
            o1T_eb = []
            for eb in range(DB):
                t = big.tile([P, S], bf16, name=f"o1T_eb{eb}")
                o1T_eb.append(t)

            def d_group(h, eb):
                ps = psA.tile([P, NF], f32, name="psD_t", tag="psA")
                for db in range(DB):
                    nc.tensor.matmul(
                        ps[:],
                        w1th(db)[:, eb * P:(eb + 1) * P],
                        gFT[:, db * S + h * NF: db * S + (h + 1) * NF],
                        start=(db == 0),
                        stop=(db == DB - 1),
                    )
                nc.scalar.activation(
                    o1T_eb[eb][:, h * NF:(h + 1) * NF], ps[:], AF.Relu,
                    bias=b1col(eb), scale=1.0)

            # ---- phase E: o2, final, normalize, store --------------------
            def emit_ot_store(sb, fin, rr):
                ot = work.tile([P, D], f32, name="ot_t", tag="ot")
                nc.vector.tensor_scalar_mul(ot[:], fin[:], rr[:])
                if sb < SB - 1:
                    nc.sync.dma_start(out[sb * P:(sb + 1) * P, :], ot[:])
                else:
                    # final block: two half stores stream in parallel
                    HD = D // 2
                    nc.sync.dma_start(out[sb * P:(sb + 1) * P, :HD],
                                      ot[:, :HD])
                    nc.scalar.dma_start(out[sb * P:(sb + 1) * P, HD:],
                                        ot[:, HD:])

            # ot+store for group k is emitted ~3 groups late: DVE is FIFO,
            # so the stt (which recycles the PSUM bank) stays ahead of the
            # deferrable scale work
            pend = []

            def e_group(sb):
                # bias matmul last: a K=128 matmul leads the group so its
                # LDWEIGHTS prefetches during the previous group's tail
                ps = psA.tile([P, NF], f32, name="psE_t", tag="psA")
                for eb in range(DB):
                    nc.tensor.matmul(
                        ps[:],
                        o1T_eb[eb][:, sb * P:(sb + 1) * P],
                        w2t(eb),
                        start=(eb == 0),
                        stop=False,
                    )
                nc.tensor.matmul(ps[:], onesrow, b2row, start=False, stop=True)
                # epilogue: fin bf16 (~1e-3 rel err); DVE: stt+recip+ot,
                # ACT: square+sqrt
                fin = work.tile([P, D], bf16, name="fin_t", tag="fin")
                sq = work.tile([P, D], bf16, name="sq_t", tag="sq")
                ss = work.tile([P, 1], f32, name="ss_t", tag="ss")
                nc.vector.scalar_tensor_tensor(
                    fin[:], ps[:], 0.0, F2_sb[sb][:], ALU.max, ALU.add)
                nc.scalar.activation(sq[:], fin[:], AF.Square,
                                     accum_out=ss[:])
                rn = work.tile([P, 1], f32, name="rn_t", tag="rn")
                nc.scalar.activation(rn[:], ss[:], AF.Sqrt)
                rr = work.tile([P, 1], f32, name="rr_t", tag="rr")
                nc.vector.reciprocal(rr[:], rn[:])
                pend.append((sb, fin, rr))
                if len(pend) > 3:
                    emit_ot_store(*pend.pop(0))

            # interleave: D-h0, E sb0-3 (o1T columns live in h0), D-h1,
            # E sb4-7 -- spreads the epilogue over D-h1's PE time
            for eb in range(DB):
                d_group(0, eb)
            for sb in range(4):
                e_group(sb)
            for eb in range(DB):
                d_group(1, eb)
            for sb in range(4, SB - 1):
                e_group(sb)
            # drain deferred stores while sb7's matmuls run, so the final
            # chain (stt7 -> ... -> ot7 -> store) isn't queued behind them
            for args in pend:
                emit_ot_store(*args)
            pend.clear()
            e_group(SB - 1)
            for args in pend:
                emit_ot_store(*args)

    nc.compile()
    return nc


def _get_compiled():
    global _COMPILED
    if _COMPILED is None:
        _COMPILED = _build()
    return _COMPILED


def _host_prep(inp, inp_len, aspect, w1, b1, w2, b2):
    inp = np.asarray(inp, dtype=np.float32)
    aspect = np.asarray(aspect, dtype=np.float32)
    inp_len = np.asarray(inp_len, dtype=np.float32)
    w1 = np.asarray(w1, dtype=np.float32)
    b1 = np.asarray(b1, dtype=np.float32)
    w2 = np.asarray(w2, dtype=np.float32)
    b2 = np.asarray(b2, dtype=np.float32)

    packW = np.zeros((P, W_COLS), dtype=BF16)
    w1th = (w1.T * 0.5).astype(BF16)                 # [d, e]
    w2tt = w2.T.astype(BF16)                         # [e, f]
    for db in range(DB):
        packW[:, W_W1 + db * D: W_W1 + (db + 1) * D] = \
            w1th[db * P:(db + 1) * P, :]
        packW[:, W_W2 + db * D: W_W2 + (db + 1) * D] = \
            w2tt[db * P:(db + 1) * P, :]
    packW[0, W_B2: W_B2 + D] = b2.astype(BF16)
    packW[0, W_ONES: W_ONES + P] = np.ones(P, dtype=BF16)

    packF = b1.reshape(DB, P).T.astype(np.float32)   # [P, DB]

    in_maps = []
    for bidx in range(B):
        x = inp[bidx].astype(np.float64)             # [S, D]
        a = aspect[bidx].astype(np.float64)
        ln = float(inp_len[bidx])
        scale = np.sqrt(ln)
        mask = (np.arange(S) < int(ln)).astype(np.float64)
        rowsum = a @ x.sum(axis=0)
        g = mask / (mask * rowsum + 1e-4 * scale)
        aTg2 = ((a * (2.0 * g)[:, None]).T).astype(BF16)   # [D, S]
        resm2 = (2.0 * (x + a) * mask[:, None]).astype(BF16)  # [S, D]

        pX = np.empty((P, X_COLS), dtype=BF16)
        xb = x.astype(BF16)
        for tb in range(TB):
            pX[:, tb * D:(tb + 1) * D] = xb[tb * P:(tb + 1) * P, :]

        pA = np.empty((P, A_COLS), dtype=BF16)
        HB = DB * NF
        for h in range(SH):
            for db in range(DB):
                pA[:, h * HB + db * NF:(h * HB + (db + 1) * NF)] = \
                    aTg2[db * P:(db + 1) * P, h * NF:(h + 1) * NF]

        pRT = np.empty((P, RT_COLS), dtype=BF16)
        r2T = resm2.T                                 # [D, S]
        for h in range(SH):
            for db in range(DB):
                pRT[:, h * HB + db * NF:(h * HB + (db + 1) * NF)] = \
                    r2T[db * P:(db + 1) * P, h * NF:(h + 1) * NF]

        pR = np.empty((P, R_COLS), dtype=BF16)
        for sb in range(SB):
            pR[:, sb * D:(sb + 1) * D] = resm2[sb * P:(sb + 1) * P, :]

        in_maps.append({"packX": pX, "packA": pA, "packRT": pRT,
                        "packR": pR, "packW": packW, "packF": packF})
    return in_maps


def kernel(inp, inp_len, aspect, w1, b1, w2, b2):
    from concourse.bass_utils import run_bass_kernel_spmd

    nc = _get_compiled()
    in_maps = _host_prep(inp, inp_len, aspect, w1, b1, w2, b2)
    res = run_bass_kernel_spmd(nc, in_maps, core_ids=list(range(N_CORES)))
    return np.stack([res.results[i]["out"] for i in range(N_CORES)], axis=0)
